# revision 5
# baseline (speedup 1.0000x reference)
"""Trainium2 Bass kernel for GNN link-prediction BCE loss.

loss = mean over 3M edges of BCE-with-logits(dot(h[src], h[dst]), label)
     = [ sum_pos softplus(-s) + sum_neg softplus(+s) ] / 3M

Strategy (8 NeuronCores, SPMD):
 - h is cast to fp16 and repacked into 16 segments of 31250 rows, each
   padded with a +1s special row used for padding edges so pads contribute
   exactly 0 loss (softplus(-128) == 0 after exp underflow).
 - pos and neg edges are merged into ONE stream with a per-edge sign
   sigma (+1 for neg edges, -1 for pos edges and pads); per-edge loss is
   ln(1+exp(-|s|)) + relu(sigma*s), accumulated on the scalar engine.
 - Edges are bucketed by (src_seg, dst_seg) on the host, distributed
   round-robin across the 8 cores, padded to a shared per-bucket quota
   (multiple of 128) so every core runs the identical instruction
   sequence on its own data.
 - The edge stream is processed in 4096-edge macro-tiles: dma_gather
   (int16 local indices) fills src/dst row tiles (gathers chunked per
   segment run, up to 4096 indices per call to amortize the ~1us SWDGE
   fixed overhead); DVE multiplies and reduces to fp16 scores; at the
   end the scalar engine computes the stable softplus with free-dim
   accumulation and PE reduces across partitions with a ones matmul.
 - Host sums the 8 partial sums and divides by 3M.
"""
import sys
sys.path.insert(0, "/opt/trn_rl_repo")
import numpy as np

import concourse.bacc as bacc
import concourse.bass as bass
import concourse.mybir as mybir
from concourse.tile import TileContext
from concourse.vector_clock import ScopedClock
from concourse.bass_utils import run_bass_kernel_spmd

N_NODES = 500_000
D = 128
E_POS = 1_000_000
E_NEG = 2_000_000
N_CORES = 8
NSEG = 16
SEG = N_NODES // NSEG            # 31250 real rows per segment
SEG_PAD = SEG + 1                # + special row (+1s at SEG)
SP = SEG                         # local index of the +1 row
MACRO = 4096                     # edges per macro tile
# Max indices per dma_gather call: each of the 16 DMA engines has a 64-slot
# in-flight descriptor ring, so num_idxs > 1024 crashes on hardware
# (verified: 1024 works; 1280, 1536, 1920, 2048 all fail).
GMAX = 1024

f16 = mybir.dt.float16
i16 = mybir.dt.int16
f32 = mybir.dt.float32

_MAX_DRAIN_WAITS = 1


class _SafeTileContext(TileContext):
    """Tail drain emits one wait per instruction (walrus rejects >2)."""

    def _drain_and_barrier(self, tick_clock, wait_clock):
        nc = self.nc
        probe = nc.sync.nop()
        wait_clock.add_sem_waits(
            probe.ins, ScopedClock({None: tick_clock.global_clock})
        )
        si = probe.ins.sync_info
        waits = list(si.on_wait or []) if si else []
        if len(waits) > _MAX_DRAIN_WAITS:
            by_name = {h.name: h for h in self.sems.allocated().values()}
            si.on_wait = []
            probe.ins.sync_info = si
            for sw in waits:
                w = nc.sync.nop(nofuse=True)
                w.wait_op(by_name[sw.ant_name], sw.wait_value, "sem-ge")
        nc.sync.drain()
        nc.all_engine_barrier()
        assert self.sems is not None
        popped = nc._tile_sem_poison_stack.pop()
        assert popped is self._sem_poison
        nc.clear_and_free_semaphores(list(self.sems.allocated().values()))
        nc.all_engine_barrier()


def _wrap16(flat):
    """[n] int16 -> [128, n//16] wrapped-in-16, replicated across Q7 cores."""
    n = flat.shape[0]
    blk = flat.reshape(n // 16, 16).T
    return np.ascontiguousarray(np.tile(blk, (8, 1)).astype(np.int16))


def _wrap128(flat, dtype):
    """[n] -> [128, n//128] with slot e at (e%128, e//128)."""
    n = flat.shape[0]
    return np.ascontiguousarray(flat.reshape(n // 128, 128).T.astype(dtype))


def _plan():
    """Shared (data-independent given quotas) op list comes from _ops; this
    function builds per-core slot arrays. Returns (quotas, per-core arrays)."""


def _assign(src, dst, sigma):
    """Bucket all edges by (src_seg, dst_seg); round-robin across cores; pad
    each bucket to a shared quota (multiple of 128).

    Returns (quota[int per bucket], per-core src_local[int16],
    dst_local[int16], sig[float16]) — all slot arrays have length n_tot.
    """
    s_seg = src // SEG
    d_seg = dst // SEG
    bucket = (s_seg * NSEG + d_seg).astype(np.int32)
    order = np.argsort(bucket, kind="stable")
    src_o = (src[order] % SEG).astype(np.int16)
    dst_o = (dst[order] % SEG).astype(np.int16)
    sig_o = sigma[order]
    bucket_o = bucket[order]
    counts = np.bincount(bucket_o, minlength=NSEG * NSEG)

    # rank within bucket -> core + position
    starts = np.zeros(NSEG * NSEG, dtype=np.int64)
    np.cumsum(counts[:-1], out=starts[1:])
    rank = np.arange(src_o.shape[0], dtype=np.int64) - starts[bucket_o]
    core = (rank % N_CORES).astype(np.int32)
    pos_in_core = rank // N_CORES

    per_core_max = (counts + N_CORES - 1) // N_CORES
    quota = ((per_core_max + 127) // 128) * 128
    qstarts = np.zeros(NSEG * NSEG, dtype=np.int64)
    np.cumsum(quota[:-1], out=qstarts[1:])
    n_tot = int(quota.sum())

    slot = qstarts[bucket_o] + pos_in_core
    src_locals = []
    dst_locals = []
    sigs = []
    for c in range(N_CORES):
        sl = np.full(n_tot, SP, dtype=np.int16)
        dl = np.full(n_tot, SP, dtype=np.int16)
        sg = np.full(n_tot, -1.0, dtype=np.float16)
        m = core == c
        sc = slot[m]
        sl[sc] = src_o[m]
        dl[sc] = dst_o[m]
        sg[sc] = sig_o[m]
        src_locals.append(sl)
        dst_locals.append(dl)
        sigs.append(sg)
    return quota, n_tot, src_locals, dst_locals, sigs


def _ops(quota):
    """Build the gather op list from bucket quotas.

    Returns (n_tot, macros) where macros is a list of
    (m_start, m_n, src_ops, dst_ops); each op is (seg, start, n) with
    start absolute in the edge-slot space.
    """
    qs = np.zeros(NSEG * NSEG + 1, dtype=np.int64)
    np.cumsum(quota, out=qs[1:])
    n_tot = int(qs[-1])

    # src runs: contiguous slot ranges with constant src_seg
    src_bounds = [int(qs[i * NSEG]) for i in range(NSEG + 1)]
    # dst runs: per-bucket ranges
    dst_bounds = qs

    def cut(ranges_starts, seg_of, m0, m1, cap):
        """ops covering [m0, m1) split at run boundaries and cap."""
        ops = []
        idx = np.searchsorted(ranges_starts, m0, side="right") - 1
        pos = m0
        while pos < m1:
            run_end = int(ranges_starts[idx + 1])
            end = min(run_end, m1)
            seg = seg_of(idx)
            while pos < end:
                n = min(cap, end - pos)
                ops.append((seg, pos, n))
                pos += n
            idx += 1
        return ops

    macros = []
    m0 = 0
    while m0 < n_tot:
        m1 = min(m0 + MACRO, n_tot)
        sops = cut(np.asarray(src_bounds), lambda i: i, m0, m1, GMAX)
        dops = cut(dst_bounds, lambda b: b % NSEG, m0, m1, GMAX)
        # drop zero-quota buckets (n == 0 can't happen; runs with equal
        # consecutive starts produce nothing because pos advances past them)
        macros.append((m0, m1 - m0, sops, dops))
        m0 = m1
    return n_tot, macros


def _build_program(quota, repeat=1):
    n_tot, macros = _ops(quota)
    w16 = n_tot // 16
    cols = n_tot // 128

    nc = bacc.Bacc("TRN2", target_bir_lowering=False)
    h16 = nc.dram_tensor("h16", [NSEG * SEG_PAD, D], f16, kind="ExternalInput")
    d_src = nc.dram_tensor("src16", [128, w16], i16, kind="ExternalInput")
    d_dst = nc.dram_tensor("dst16", [128, w16], i16, kind="ExternalInput")
    d_sig = nc.dram_tensor("sig", [128, cols], f16, kind="ExternalInput")
    out = nc.dram_tensor("partial", [1, 1], f32, kind="ExternalOutput")

    AF = mybir.ActivationFunctionType
    with _SafeTileContext(nc) as tc:
        with (
            tc.tile_pool(name="idxp", bufs=3) as idxp,
            tc.tile_pool(name="srcg", bufs=2) as srcg,
            tc.tile_pool(name="dstg", bufs=2) as dstg,
            tc.tile_pool(name="work", bufs=2) as workp,
            tc.tile_pool(name="persist", bufs=1) as persist,
            tc.tile_pool(name="fin", bufs=1) as finp,
            tc.tile_pool(name="psum", bufs=1, space="PSUM") as psump,
        ):
            nregs = {}

            def nreg(n):
                if n not in nregs:
                    nregs[n] = nc.gpsimd.snap(n)
                return nregs[n]

            sig = persist.tile([128, cols], f16, tag="sig")
            nc.sync.dma_start(out=sig[:], in_=d_sig[:, :])
            score = persist.tile([128, cols], f16, tag="score")

            for _rep in range(repeat):
                for (m0, m_n, sops, dops) in macros:
                    mcols = m_n // 128
                    si = idxp.tile([128, MACRO // 16], i16, tag="si")
                    di = idxp.tile([128, MACRO // 16], i16, tag="di")
                    nc.sync.dma_start(
                        out=si[:, :m_n // 16], in_=d_src[:, m0 // 16:(m0 + m_n) // 16])
                    nc.sync.dma_start(
                        out=di[:, :m_n // 16], in_=d_dst[:, m0 // 16:(m0 + m_n) // 16])

                    st = srcg.tile([128, MACRO], f16, tag="st")
                    dt_ = dstg.tile([128, MACRO], f16, tag="dt")
                    for (seg, start, n) in sops:
                        o = start - m0
                        nc.gpsimd.dma_gather(
                            st[:, o:o + n].rearrange("p (c d) -> p c d", d=D),
                            h16[seg * SEG_PAD:, :],
                            si[:, o // 16:(o + n) // 16],
                            n, nreg(n), D,
                        )
                    for (seg, start, n) in dops:
                        o = start - m0
                        nc.gpsimd.dma_gather(
                            dt_[:, o:o + n].rearrange("p (c d) -> p c d", d=D),
                            h16[seg * SEG_PAD:, :],
                            di[:, o // 16:(o + n) // 16],
                            n, nreg(n), D,
                        )
                    prod = workp.tile([128, MACRO], f16, tag="prod")
                    nc.vector.tensor_tensor(
                        out=prod[:, :m_n], in0=st[:, :m_n], in1=dt_[:, :m_n],
                        op=mybir.AluOpType.mult,
                    )
                    with nc.allow_low_precision(
                            reason="fp16 score ok for 2e-2 tolerance"):
                        nc.vector.tensor_reduce(
                            out=score[:, m0 // 128:m0 // 128 + mcols],
                            in_=prod[:, :m_n].rearrange("p (c d) -> p c d", d=D),
                            axis=mybir.AxisListType.X,
                            op=mybir.AluOpType.add,
                        )

                # loss_e = ln(1+exp(-|s|)) + relu(sigma*s)
                acc = finp.tile([128, 2], f32, tag="acc")
                t1 = finp.tile([128, cols], f16, tag="sp_t1")
                t2 = finp.tile([128, cols], f16, tag="sp_t2")
                nc.scalar.activation(out=t1[:], in_=score[:], func=AF.Abs)
                nc.scalar.activation(out=t2[:], in_=t1[:], func=AF.Exp, scale=-1.0)
                nc.scalar.activation(out=t1[:], in_=t2[:], func=AF.Ln, bias=1.0,
                                     accum_out=acc[:, 0:1])
                nc.vector.tensor_tensor(out=t2[:], in0=score[:], in1=sig[:],
                                        op=mybir.AluOpType.mult)
                nc.scalar.activation(out=t2[:], in_=t2[:], func=AF.Relu,
                                     accum_out=acc[:, 1:2])

                acc1 = finp.tile([128, 1], f32, tag="acc1")
                nc.vector.tensor_reduce(out=acc1[:], in_=acc[:],
                                        axis=mybir.AxisListType.X,
                                        op=mybir.AluOpType.add)
                ones = finp.tile([128, 1], f32, tag="ones")
                nc.vector.memset(ones[:], 1.0)
                ps = psump.tile([1, 1], f32, tag="ps")
                nc.tensor.matmul(ps[:], lhsT=acc1[:], rhs=ones[:],
                                 start=True, stop=True)
                res = finp.tile([1, 1], f32, tag="res")
                nc.vector.tensor_copy(out=res[:], in_=ps[:])
            nc.sync.dma_start(out=out[:, :], in_=res[:])
    nc.finalize()
    return nc


def _pack_table(h):
    """fp32 [N, D] -> fp16 [NSEG*SEG_PAD, D] with +1 special rows."""
    t = np.empty((NSEG * SEG_PAD, D), dtype=np.float16)
    hv = h.astype(np.float16).reshape(NSEG, SEG, D)
    for s in range(NSEG):
        t[s * SEG_PAD:s * SEG_PAD + SEG] = hv[s]
        t[s * SEG_PAD + SP] = np.float16(1.0)
    return t


def _prepare(h, pos_src, pos_dst, neg_src, neg_dst):
    """Host-side planning: returns (quota, in_maps)."""
    h = np.asarray(h)
    src = np.concatenate([np.asarray(pos_src), np.asarray(neg_src)]).astype(np.int64)
    dst = np.concatenate([np.asarray(pos_dst), np.asarray(neg_dst)]).astype(np.int64)
    sigma = np.concatenate([
        np.full(E_POS, -1.0, dtype=np.float16),
        np.full(E_NEG, +1.0, dtype=np.float16),
    ])

    table = _pack_table(h)
    quota, n_tot, src_l, dst_l, sig_l = _assign(src, dst, sigma)

    in_maps = []
    for c in range(N_CORES):
        in_maps.append({
            "h16": table,
            "src16": _wrap16(src_l[c]),
            "dst16": _wrap16(dst_l[c]),
            "sig": _wrap128(sig_l[c], np.float16),
        })
    return quota, in_maps


def kernel(h, pos_src, pos_dst, neg_src, neg_dst):
    quota, in_maps = _prepare(h, pos_src, pos_dst, neg_src, neg_dst)
    nc = _build_program(quota)
    res = run_bass_kernel_spmd(nc, in_maps, core_ids=list(range(N_CORES)))
    total = float(sum(float(r["partial"][0, 0]) for r in res.results))
    loss = total / float(E_POS + E_NEG)
    return np.float32(loss)


if __name__ == "__main__":
    rng = np.random.default_rng(0)
    h = rng.standard_normal((N_NODES, D)).astype(np.float32)
    a = rng.integers(0, N_NODES, size=E_POS)
    b = rng.integers(0, N_NODES, size=E_POS)
    c_ = rng.integers(0, N_NODES, size=E_NEG)
    d_ = rng.integers(0, N_NODES, size=E_NEG)
    got = kernel(h, a, b, c_, d_)
    s1 = np.einsum("ij,ij->i", h[a].astype(np.float32), h[b].astype(np.float32))
    s2 = np.einsum("ij,ij->i", h[c_].astype(np.float32), h[d_].astype(np.float32))
    exp = (np.logaddexp(0, -s1).sum() + np.logaddexp(0, s2).sum()) / 3e6
    print("got", got, "exp", exp, "rel", abs(got - exp) / abs(exp))


# revision 7
# speedup vs baseline: 4.0557x; 4.0557x over previous
"""Trainium2 Bass kernel for GNN link-prediction BCE loss.

loss = mean over 3M edges of BCE-with-logits(dot(h[src], h[dst]), label)
     = [ sum_pos softplus(-s) + sum_neg softplus(+s) ] / 3M

Strategy (8 NeuronCores, SPMD):
 - h is cast to fp16 and repacked into 16 segments of 31250 rows, each
   padded with a +1s special row used for padding edges so pads contribute
   exactly 0 loss (softplus(-128) == 0 after exp underflow).
 - pos and neg edges are merged into ONE stream with a per-edge sign
   sigma (+1 for neg edges, -1 for pos edges and pads); per-edge loss is
   ln(1+exp(-|s|)) + relu(sigma*s), accumulated on the scalar engine.
 - Edges are bucketed by (src_seg, dst_seg) on the host, distributed
   round-robin across the 8 cores, padded to a shared per-bucket quota
   (multiple of 128) so every core runs the identical instruction
   sequence on its own data.
 - The edge stream is processed in 4096-edge macro-tiles: dma_gather
   (int16 local indices) fills src/dst row tiles (gathers chunked per
   segment run, up to 4096 indices per call to amortize the ~1us SWDGE
   fixed overhead); DVE multiplies and reduces to fp16 scores; at the
   end the scalar engine computes the stable softplus with free-dim
   accumulation and PE reduces across partitions with a ones matmul.
 - Host sums the 8 partial sums and divides by 3M.
"""
import sys
sys.path.insert(0, "/opt/trn_rl_repo")
import numpy as np

import concourse.bacc as bacc
import concourse.bass as bass
import concourse.mybir as mybir
from concourse.tile import TileContext
from concourse.vector_clock import ScopedClock
from concourse.bass_utils import run_bass_kernel_spmd

N_NODES = 500_000
D = 128
E_POS = 1_000_000
E_NEG = 2_000_000
N_CORES = 8
NSEG = 16
SEG = N_NODES // NSEG            # 31250 real rows per segment
SEG_PAD = SEG + 1                # + special row (+1s at SEG)
SP = SEG                         # local index of the +1 row
MACRO = 4096                     # edges per macro tile
# Max indices per dma_gather call: each of the 16 DMA engines has a 64-slot
# in-flight descriptor ring, so num_idxs > 1024 crashes on hardware
# (verified: 1024 works; 1280, 1536, 1920, 2048 all fail).
GMAX = 1024

f16 = mybir.dt.float16
i16 = mybir.dt.int16
f32 = mybir.dt.float32

_MAX_DRAIN_WAITS = 1


class _SafeTileContext(TileContext):
    """Tail drain emits one wait per instruction (walrus rejects >2)."""

    def _drain_and_barrier(self, tick_clock, wait_clock):
        nc = self.nc
        probe = nc.sync.nop()
        wait_clock.add_sem_waits(
            probe.ins, ScopedClock({None: tick_clock.global_clock})
        )
        si = probe.ins.sync_info
        waits = list(si.on_wait or []) if si else []
        if len(waits) > _MAX_DRAIN_WAITS:
            by_name = {h.name: h for h in self.sems.allocated().values()}
            si.on_wait = []
            probe.ins.sync_info = si
            for sw in waits:
                w = nc.sync.nop(nofuse=True)
                w.wait_op(by_name[sw.ant_name], sw.wait_value, "sem-ge")
        nc.sync.drain()
        nc.all_engine_barrier()
        assert self.sems is not None
        popped = nc._tile_sem_poison_stack.pop()
        assert popped is self._sem_poison
        nc.clear_and_free_semaphores(list(self.sems.allocated().values()))
        nc.all_engine_barrier()


def _wrap16(flat):
    """[n] int16 -> [128, n//16] wrapped-in-16, replicated across Q7 cores."""
    n = flat.shape[0]
    blk = flat.reshape(n // 16, 16).T
    return np.ascontiguousarray(np.tile(blk, (8, 1)).astype(np.int16))


def _wrap128(flat, dtype):
    """[n] -> [128, n//128] with slot e at (e%128, e//128)."""
    n = flat.shape[0]
    return np.ascontiguousarray(flat.reshape(n // 128, 128).T.astype(dtype))


def _plan():
    """Shared (data-independent given quotas) op list comes from _ops; this
    function builds per-core slot arrays. Returns (quotas, per-core arrays)."""


def _assign(src, dst, sigma):
    """Bucket all edges by (src_seg, dst_seg); round-robin across cores; pad
    each bucket to a shared quota (multiple of 128).

    Returns (quota[int per bucket], per-core src_local[int16],
    dst_local[int16], sig[float16]) — all slot arrays have length n_tot.
    """
    s_seg = src // SEG
    d_seg = dst // SEG
    bucket = (s_seg * NSEG + d_seg).astype(np.int32)
    order = np.argsort(bucket, kind="stable")
    src_o = (src[order] % SEG).astype(np.int16)
    dst_o = (dst[order] % SEG).astype(np.int16)
    sig_o = sigma[order]
    bucket_o = bucket[order]
    counts = np.bincount(bucket_o, minlength=NSEG * NSEG)

    # rank within bucket -> core + position
    starts = np.zeros(NSEG * NSEG, dtype=np.int64)
    np.cumsum(counts[:-1], out=starts[1:])
    rank = np.arange(src_o.shape[0], dtype=np.int64) - starts[bucket_o]
    core = (rank % N_CORES).astype(np.int32)
    pos_in_core = rank // N_CORES

    per_core_max = (counts + N_CORES - 1) // N_CORES
    quota = ((per_core_max + 127) // 128) * 128
    qstarts = np.zeros(NSEG * NSEG, dtype=np.int64)
    np.cumsum(quota[:-1], out=qstarts[1:])
    n_tot = int(quota.sum())

    slot = qstarts[bucket_o] + pos_in_core
    src_locals = []
    dst_locals = []
    sigs = []
    for c in range(N_CORES):
        sl = np.full(n_tot, SP, dtype=np.int16)
        dl = np.full(n_tot, SP, dtype=np.int16)
        sg = np.full(n_tot, -1.0, dtype=np.float16)
        m = core == c
        sc = slot[m]
        sl[sc] = src_o[m]
        dl[sc] = dst_o[m]
        sg[sc] = sig_o[m]
        src_locals.append(sl)
        dst_locals.append(dl)
        sigs.append(sg)
    return quota, n_tot, src_locals, dst_locals, sigs


def _ops(quota):
    """Build the gather op list from bucket quotas.

    Returns (n_tot, macros) where macros is a list of
    (m_start, m_n, src_ops, dst_ops); each op is (seg, start, n) with
    start absolute in the edge-slot space.
    """
    qs = np.zeros(NSEG * NSEG + 1, dtype=np.int64)
    np.cumsum(quota, out=qs[1:])
    n_tot = int(qs[-1])

    # src runs: contiguous slot ranges with constant src_seg
    src_bounds = [int(qs[i * NSEG]) for i in range(NSEG + 1)]
    # dst runs: per-bucket ranges
    dst_bounds = qs

    def cut(ranges_starts, seg_of, m0, m1, cap):
        """ops covering [m0, m1) split at run boundaries and cap."""
        ops = []
        idx = np.searchsorted(ranges_starts, m0, side="right") - 1
        pos = m0
        while pos < m1:
            run_end = int(ranges_starts[idx + 1])
            end = min(run_end, m1)
            seg = seg_of(idx)
            while pos < end:
                n = min(cap, end - pos)
                ops.append((seg, pos, n))
                pos += n
            idx += 1
        return ops

    macros = []
    m0 = 0
    while m0 < n_tot:
        m1 = min(m0 + MACRO, n_tot)
        sops = cut(np.asarray(src_bounds), lambda i: i, m0, m1, GMAX)
        dops = cut(dst_bounds, lambda b: b % NSEG, m0, m1, GMAX)
        # drop zero-quota buckets (n == 0 can't happen; runs with equal
        # consecutive starts produce nothing because pos advances past them)
        macros.append((m0, m1 - m0, sops, dops))
        m0 = m1
    return n_tot, macros


def _build_program(quota, repeat=1):
    n_tot, macros = _ops(quota)
    w16 = n_tot // 16
    cols = n_tot // 128

    nc = bacc.Bacc("TRN2", target_bir_lowering=False, num_swdge_queues=4)
    h16 = nc.dram_tensor("h16", [NSEG * SEG_PAD, D], f16, kind="ExternalInput")
    d_src = nc.dram_tensor("src16", [128, w16], i16, kind="ExternalInput")
    d_dst = nc.dram_tensor("dst16", [128, w16], i16, kind="ExternalInput")
    d_sig = nc.dram_tensor("sig", [128, cols], f16, kind="ExternalInput")
    out = nc.dram_tensor("partial", [1, 1], f32, kind="ExternalOutput")

    AF = mybir.ActivationFunctionType
    with _SafeTileContext(nc) as tc:
        with (
            tc.tile_pool(name="idxp", bufs=3) as idxp,
            tc.tile_pool(name="srcg", bufs=2) as srcg,
            tc.tile_pool(name="dstg", bufs=2) as dstg,
            tc.tile_pool(name="work", bufs=2) as workp,
            tc.tile_pool(name="persist", bufs=1) as persist,
            tc.tile_pool(name="fin", bufs=1) as finp,
            tc.tile_pool(name="psum", bufs=1, space="PSUM") as psump,
        ):
            nregs = {}

            def nreg(n):
                if n not in nregs:
                    nregs[n] = nc.gpsimd.snap(n)
                return nregs[n]

            qctr = [0]

            def nextq():
                qctr[0] = (qctr[0] + 1) % 4
                return qctr[0]

            sig = persist.tile([128, cols], f16, tag="sig")
            nc.sync.dma_start(out=sig[:], in_=d_sig[:, :])
            score = persist.tile([128, cols], f16, tag="score")

            for _rep in range(repeat):
                for (m0, m_n, sops, dops) in macros:
                    mcols = m_n // 128
                    si = idxp.tile([128, MACRO // 16], i16, tag="si")
                    di = idxp.tile([128, MACRO // 16], i16, tag="di")
                    nc.sync.dma_start(
                        out=si[:, :m_n // 16], in_=d_src[:, m0 // 16:(m0 + m_n) // 16])
                    nc.sync.dma_start(
                        out=di[:, :m_n // 16], in_=d_dst[:, m0 // 16:(m0 + m_n) // 16])

                    st = srcg.tile([128, MACRO], f16, tag="st")
                    dt_ = dstg.tile([128, MACRO], f16, tag="dt")
                    for (seg, start, n) in sops:
                        o = start - m0
                        nc.gpsimd.dma_gather(
                            st[:, o:o + n].rearrange("p (c d) -> p c d", d=D),
                            h16[seg * SEG_PAD:, :],
                            si[:, o // 16:(o + n) // 16],
                            n, nreg(n), D, queue_num=nextq(),
                        )
                    for (seg, start, n) in dops:
                        o = start - m0
                        nc.gpsimd.dma_gather(
                            dt_[:, o:o + n].rearrange("p (c d) -> p c d", d=D),
                            h16[seg * SEG_PAD:, :],
                            di[:, o // 16:(o + n) // 16],
                            n, nreg(n), D, queue_num=nextq(),
                        )
                    prod = workp.tile([128, MACRO], f16, tag="prod")
                    nc.vector.tensor_tensor(
                        out=prod[:, :m_n], in0=st[:, :m_n], in1=dt_[:, :m_n],
                        op=mybir.AluOpType.mult,
                    )
                    with nc.allow_low_precision(
                            reason="fp16 score ok for 2e-2 tolerance"):
                        nc.vector.tensor_reduce(
                            out=score[:, m0 // 128:m0 // 128 + mcols],
                            in_=prod[:, :m_n].rearrange("p (c d) -> p c d", d=D),
                            axis=mybir.AxisListType.X,
                            op=mybir.AluOpType.add,
                        )

                # loss_e = ln(1+exp(-|s|)) + relu(sigma*s)
                acc = finp.tile([128, 2], f32, tag="acc")
                t1 = finp.tile([128, cols], f16, tag="sp_t1")
                t2 = finp.tile([128, cols], f16, tag="sp_t2")
                nc.scalar.activation(out=t1[:], in_=score[:], func=AF.Abs)
                nc.scalar.activation(out=t2[:], in_=t1[:], func=AF.Exp, scale=-1.0)
                nc.scalar.activation(out=t1[:], in_=t2[:], func=AF.Ln, bias=1.0,
                                     accum_out=acc[:, 0:1])
                nc.vector.tensor_tensor(out=t2[:], in0=score[:], in1=sig[:],
                                        op=mybir.AluOpType.mult)
                nc.scalar.activation(out=t2[:], in_=t2[:], func=AF.Relu,
                                     accum_out=acc[:, 1:2])

                acc1 = finp.tile([128, 1], f32, tag="acc1")
                nc.vector.tensor_reduce(out=acc1[:], in_=acc[:],
                                        axis=mybir.AxisListType.X,
                                        op=mybir.AluOpType.add)
                ones = finp.tile([128, 1], f32, tag="ones")
                nc.vector.memset(ones[:], 1.0)
                ps = psump.tile([1, 1], f32, tag="ps")
                nc.tensor.matmul(ps[:], lhsT=acc1[:], rhs=ones[:],
                                 start=True, stop=True)
                res = finp.tile([1, 1], f32, tag="res")
                nc.vector.tensor_copy(out=res[:], in_=ps[:])
            nc.sync.dma_start(out=out[:, :], in_=res[:])
    nc.finalize()
    return nc


def _pack_table(h):
    """fp32 [N, D] -> fp16 [NSEG*SEG_PAD, D] with +1 special rows."""
    t = np.empty((NSEG * SEG_PAD, D), dtype=np.float16)
    hv = h.astype(np.float16).reshape(NSEG, SEG, D)
    for s in range(NSEG):
        t[s * SEG_PAD:s * SEG_PAD + SEG] = hv[s]
        t[s * SEG_PAD + SP] = np.float16(1.0)
    return t


def _prepare(h, pos_src, pos_dst, neg_src, neg_dst):
    """Host-side planning: returns (quota, in_maps)."""
    h = np.asarray(h)
    src = np.concatenate([np.asarray(pos_src), np.asarray(neg_src)]).astype(np.int64)
    dst = np.concatenate([np.asarray(pos_dst), np.asarray(neg_dst)]).astype(np.int64)
    sigma = np.concatenate([
        np.full(E_POS, -1.0, dtype=np.float16),
        np.full(E_NEG, +1.0, dtype=np.float16),
    ])

    table = _pack_table(h)
    quota, n_tot, src_l, dst_l, sig_l = _assign(src, dst, sigma)

    in_maps = []
    for c in range(N_CORES):
        in_maps.append({
            "h16": table,
            "src16": _wrap16(src_l[c]),
            "dst16": _wrap16(dst_l[c]),
            "sig": _wrap128(sig_l[c], np.float16),
        })
    return quota, in_maps


def kernel(h, pos_src, pos_dst, neg_src, neg_dst):
    quota, in_maps = _prepare(h, pos_src, pos_dst, neg_src, neg_dst)
    nc = _build_program(quota)
    res = run_bass_kernel_spmd(nc, in_maps, core_ids=list(range(N_CORES)))
    total = float(sum(float(r["partial"][0, 0]) for r in res.results))
    loss = total / float(E_POS + E_NEG)
    return np.float32(loss)


if __name__ == "__main__":
    rng = np.random.default_rng(0)
    h = rng.standard_normal((N_NODES, D)).astype(np.float32)
    a = rng.integers(0, N_NODES, size=E_POS)
    b = rng.integers(0, N_NODES, size=E_POS)
    c_ = rng.integers(0, N_NODES, size=E_NEG)
    d_ = rng.integers(0, N_NODES, size=E_NEG)
    got = kernel(h, a, b, c_, d_)
    s1 = np.einsum("ij,ij->i", h[a].astype(np.float32), h[b].astype(np.float32))
    s2 = np.einsum("ij,ij->i", h[c_].astype(np.float32), h[d_].astype(np.float32))
    exp = (np.logaddexp(0, -s1).sum() + np.logaddexp(0, s2).sum()) / 3e6
    print("got", got, "exp", exp, "rel", abs(got - exp) / abs(exp))
